# revision 1
# baseline (speedup 1.0000x reference)
"""MoE adapter kernel for 8 Trainium2 NeuronCores — v5.

Same device program as v2 (fp16 slots + full-fp8 DoubleRow L1 slots,
theta=0.48 gate-weight eligibility, W1 2^6 pre-scale, interleaved slot
order for the power limiter) with an ns-exact solver cost model:
fp16 slot = 1280*(s/2.4+2.5) + 512*(s/2.4+2.5); fp8 L1 pair cost
max(213.3, 0.4708*s) (DoubleRow is LDWEIGHTS-bound below N~453, so fp8
slots are constrained to >= 352 tokens).
"""

import os
import numpy as np
import ml_dtypes

B = 8192
IN_DIM = 5120
HID = 4096
OUT_DIM = 2048
E = 8
NCORES = 8
KT1 = IN_DIM // 128
HT = HID // 128
KT2 = HID // 128
OT = OUT_DIM // 128

THETA = 0.48
W1_SCALE = 64.0

LAST_RESULT = None

import random as _random

_MAXS = 512
_MINS16 = 288
_MINS8 = 352
_SLOT_PEN = 5000.0


def _minsz(t):
    return _MINS16 if t == "16" else _MINS8


def _cost_slot(t, s):
    mm16 = s / 2.4 + 2.5
    l2 = 512 * mm16
    if t == "16":
        comp = 1280 * mm16 + l2
        dma = 209000.0
    else:
        comp = 640 * max(213.3, 0.4708 * s) + l2
        dma = 134000.0
    return max(comp, dma) + _SLOT_PEN


def _coverage(slots, n, e, elig):
    cap16 = cap8 = 0
    for (t, s), k in zip(slots, n[e]):
        if t == "16":
            cap16 += s * k
        else:
            cap8 += s * k
    return cap16 + min(cap8, elig[e])


def _makespan(slots, n, counts, elig):
    cost = sum(_cost_slot(t, s) for (t, s) in slots)
    pen = 0.0
    for e in range(E):
        deficit = counts[e] - _coverage(slots, n, e, elig)
        if deficit > 0:
            pen += 50000.0 + deficit * 7000.0
    for i in range(len(slots)):
        used = sum(n[e][i] for e in range(E))
        if used > NCORES:
            pen += (used - NCORES) * 3e6
    return cost + pen


def _greedy16(counts):
    slots = []
    n = [[] for _ in range(E)]
    need = list(counts)
    while any(x > 0 for x in need):
        slots.append(("16", 448))
        for e in range(E):
            n[e].append(0)
        copies = NCORES
        for e in sorted(range(E), key=lambda e: -need[e]):
            while need[e] > 0 and copies > 0:
                n[e][-1] += 1
                need[e] -= 448
                copies -= 1
    return slots, n


def _solve_once(counts, elig, iters, seed):
    rng = _random.Random(seed)
    slots, n = _greedy16(counts)
    cur = _makespan(slots, n, counts, elig)
    best = (cur, [s for s in slots], [r[:] for r in n])
    for it in range(iters):
        T = max(0.02, 1.0 * (1 - it / iters))
        op = rng.random()
        slots2 = [s for s in slots]
        n2 = [r[:] for r in n]
        if op < 0.35 and slots2:
            i = rng.randrange(len(slots2))
            t, s = slots2[i]
            s2 = min(_MAXS, max(_minsz(t),
                                s + rng.choice([-64, -32, -16, 16, 32, 64])))
            slots2[i] = (t, s2)
        elif op < 0.55 and slots2:
            i = rng.randrange(len(slots2))
            e1 = rng.randrange(E)
            if n2[e1][i] > 0:
                n2[e1][i] -= 1
                if rng.random() < 0.8:
                    n2[rng.randrange(E)][i] += 1
        elif op < 0.70 and slots2:
            n2[rng.randrange(E)][rng.randrange(len(slots2))] += 1
        elif op < 0.80 and slots2:
            i = rng.randrange(len(slots2))
            t, s = slots2[i]
            t2 = "8" if t == "16" else "16"
            slots2[i] = (t2, max(s, _minsz(t2)))
        elif op < 0.90 and len(slots2) > 1:
            i = rng.randrange(len(slots2))
            del slots2[i]
            for e in range(E):
                del n2[e][i]
        else:
            t2 = rng.choice(["16", "8"])
            slots2.append((t2, rng.choice(range(_minsz(t2), _MAXS + 1, 16))))
            for e in range(E):
                n2[e].append(0)
            i = len(slots2) - 1
            copies = NCORES
            for e in sorted(range(E), key=lambda e: -counts[e])[:NCORES]:
                if copies:
                    n2[e][i] += 1
                    copies -= 1
        c2 = _makespan(slots2, n2, counts, elig)
        if c2 <= cur or rng.random() < pow(2.718, -(c2 - cur) / (T * 30000)):
            slots, n, cur = slots2, n2, c2
            if cur < best[0]:
                best = (cur, [s for s in slots], [r[:] for r in n])
    return best


def _solve(counts, elig, restarts=10, iters=200000):
    best = None
    for r in range(restarts):
        c, slots, n = _solve_once(counts, elig, iters, seed=r)
        if best is None or c < best[0]:
            best = (c, slots, n)
    c, slots, n = best
    keep = [i for i in range(len(slots)) if any(n[e][i] for e in range(E))]
    slots = [slots[i] for i in keep]
    n = [[r[i] for i in keep] for r in n]

    # repair any residual deficit deterministically
    for _ in range(64):
        bad = [e for e in range(E)
               if _coverage(slots, n, e, elig) < counts[e]]
        if not bad:
            break
        e = bad[0]
        grown = False
        for i in range(len(slots)):
            t, s = slots[i]
            if n[e][i] and s + 16 <= _MAXS:
                slots[i] = (t, s + 16)
                grown = True
                break
        if not grown:
            for i in range(len(slots)):
                if sum(n[x][i] for x in range(E)) < NCORES:
                    n[e][i] += 1
                    grown = True
                    break
        if not grown:
            slots.append(("16", 448))
            for x in range(E):
                n[x].append(1 if x == e else 0)
    if any(_coverage(slots, n, e, elig) < counts[e] for e in range(E)):
        slots, n = _greedy16(counts)

    # interleave fp8 slots between fp16 slots (power limiter)
    f16 = sorted([i for i in range(len(slots)) if slots[i][0] == "16"],
                 key=lambda i: -slots[i][1])
    f8 = sorted([i for i in range(len(slots)) if slots[i][0] == "8"],
                key=lambda i: -slots[i][1])
    order = []
    while f16 or f8:
        if f16:
            order.append(f16.pop(0))
        if f8:
            order.append(f8.pop(0))
    slots = [slots[i] for i in order]
    n = [[r[i] for i in order] for r in n]
    return slots, n


_NC = {}


def _build_bass(slot_key):
    import concourse.mybir as mybir
    import concourse.tile as tile
    from concourse import bacc
    from concourse.bass import ts

    f16 = mybir.dt.float16
    f32 = mybir.dt.float32
    f8 = mybir.dt.float8e4
    DR = mybir.MatmulPerfMode.DoubleRow
    relu = mybir.ActivationFunctionType.Relu

    nc = bacc.Bacc("TRN2", target_bir_lowering=False, debug=False,
                   num_devices=NCORES)

    slots = list(slot_key)
    dram = []
    for s, (t, sz) in enumerate(slots):
        if t == "16":
            xt_d = nc.dram_tensor(f"xt_{s}", [128, KT1 * sz], f16,
                                  kind="ExternalInput")
            w1_d = nc.dram_tensor(f"w1_{s}", [HT, 128, KT1 * 128], f16,
                                  kind="ExternalInput")
        else:
            xt_d = nc.dram_tensor(f"xt_{s}", [128, KT1, sz], f8,
                                  kind="ExternalInput")
            w1_d = nc.dram_tensor(f"w1_{s}", [HT, 128, KT1, 128], f8,
                                  kind="ExternalInput")
        w2_d = nc.dram_tensor(f"w2_{s}", [OT, 128, KT2 * 128], f16,
                              kind="ExternalInput")
        b1_d = nc.dram_tensor(f"b1_{s}", [128, HT], f32,
                              kind="ExternalInput")
        yt_d = nc.dram_tensor(f"yt_{s}", [OT, 128, sz], f32,
                              kind="ExternalOutput")
        dram.append((xt_d, w1_d, w2_d, b1_d, yt_d))

    max16 = max([sz for (t, sz) in slots if t == "16"], default=0)
    wbufs = 3 if max16 <= 460 else 2

    with tile.TileContext(nc) as tc:
        with (
            tc.tile_pool(name="xt", bufs=2) as xt_pool,
            tc.tile_pool(name="w1", bufs=wbufs) as w1_pool,
            tc.tile_pool(name="w2", bufs=wbufs) as w2_pool,
            tc.tile_pool(name="h", bufs=2) as h_pool,
            tc.tile_pool(name="b", bufs=2) as b_pool,
            tc.tile_pool(name="y", bufs=4) as y_pool,
            tc.tile_pool(name="ps1", bufs=2, space="PSUM") as ps1_pool,
            tc.tile_pool(name="ps2", bufs=2, space="PSUM") as ps2_pool,
        ):
            for s, (t, sz) in enumerate(slots):
                xt_d, w1_d, w2_d, b1_d, yt_d = dram[s]
                b1t = b_pool.tile([128, HT], f32, tag="b1")
                nc.sync.dma_start(out=b1t[:], in_=b1_d.ap())
                h_sb = h_pool.tile([128, HT * sz], f16, tag="h")

                if t == "16":
                    xt = xt_pool.tile([128, KT1 * sz], f16, tag="xt")
                    for h in range(HT):
                        w1t = w1_pool.tile([128, KT1 * 128], f16, tag="w1")
                        nc.sync.dma_start(out=w1t[:], in_=w1_d.ap()[h])
                        ps = ps1_pool.tile([128, sz], f32, tag="ps1")
                        for k in range(KT1):
                            if h == 0 and k % 10 == 0:
                                cols = slice(k * sz, (k + 10) * sz)
                                nc.sync.dma_start(out=xt[:, cols],
                                                  in_=xt_d.ap()[:, cols])
                            nc.tensor.matmul(ps[:], w1t[:, ts(k, 128)],
                                             xt[:, ts(k, sz)],
                                             start=(k == 0),
                                             stop=(k == KT1 - 1))
                        nc.scalar.activation(h_sb[:, ts(h, sz)], ps[:], relu,
                                             bias=b1t[:, h:h + 1])
                else:
                    xt = xt_pool.tile([128, KT1, sz], f8, tag="xt")
                    for h in range(HT):
                        w1t = w1_pool.tile([128, KT1, 128], f8, tag="w1")
                        nc.sync.dma_start(out=w1t[:], in_=w1_d.ap()[h])
                        ps = ps1_pool.tile([128, sz], f32, tag="ps1")
                        for p in range(KT1 // 2):
                            if h == 0 and p % 5 == 0:
                                ksl = slice(2 * p, 2 * p + 10)
                                nc.sync.dma_start(out=xt[:, ksl, :],
                                                  in_=xt_d.ap()[:, ksl, :])
                            nc.tensor.matmul(ps[:], w1t[:, 2 * p:2 * p + 2, :],
                                             xt[:, 2 * p:2 * p + 2, :],
                                             start=(p == 0),
                                             stop=(p == KT1 // 2 - 1),
                                             perf_mode=DR)
                        nc.scalar.activation(h_sb[:, ts(h, sz)], ps[:], relu,
                                             bias=b1t[:, h:h + 1],
                                             scale=1.0 / W1_SCALE)

                for o in range(OT):
                    w2t = w2_pool.tile([128, KT2 * 128], f16, tag="w2")
                    nc.sync.dma_start(out=w2t[:], in_=w2_d.ap()[o])
                    ps2 = ps2_pool.tile([128, sz], f32, tag="ps2")
                    for k in range(KT2):
                        nc.tensor.matmul(ps2[:], w2t[:, ts(k, 128)],
                                         h_sb[:, ts(k, sz)],
                                         start=(k == 0), stop=(k == KT2 - 1))
                    yt_sb = y_pool.tile([128, sz], f32, tag="y")
                    nc.vector.tensor_copy(yt_sb[:], ps2[:])
                    nc.sync.dma_start(out=yt_d.ap()[o], in_=yt_sb[:])

    nc.compile()
    return nc


def _get_nc(slot_key):
    if slot_key not in _NC:
        _NC[slot_key] = _build_bass(slot_key)
    return _NC[slot_key]


def _route(X, gW1, gb1, gW2, gb2):
    g = np.maximum(X.astype(np.float64) @ gW1.astype(np.float64)
                   + gb1.astype(np.float64), 0.0)
    logits = g @ gW2.astype(np.float64) + gb2.astype(np.float64)
    top2 = np.argpartition(-logits, 1, axis=1)[:, :2]
    l2 = np.take_along_axis(logits, top2, axis=1)
    ew = np.exp(l2 - l2.max(axis=1, keepdims=True))
    wts = ew / ew.sum(axis=1, keepdims=True)
    return top2, wts.astype(np.float32)


def _pack_x16(xb):
    blk = xb.shape[0]
    return np.ascontiguousarray(
        xb.T.reshape(KT1, 128, blk).transpose(1, 0, 2)
    ).reshape(128, KT1 * blk).astype(np.float16)


def _pack_x8(xb):
    blk = xb.shape[0]
    return np.ascontiguousarray(
        xb.T.reshape(KT1, 128, blk).transpose(1, 0, 2)
    ).astype(ml_dtypes.float8_e4m3)


def kernel(id_emb, llm_emb, W1, b1, W2, b2, gW1, gb1, gW2, gb2):
    global LAST_RESULT
    from concourse.bass_utils import run_bass_kernel_spmd

    X = np.concatenate([np.asarray(id_emb, np.float32),
                        np.asarray(llm_emb, np.float32)], axis=1)
    W1 = np.asarray(W1, np.float32); b1 = np.asarray(b1, np.float32)
    W2 = np.asarray(W2, np.float32); b2 = np.asarray(b2, np.float32)

    top2, wts = _route(X, np.asarray(gW1), np.asarray(gb1),
                       np.asarray(gW2), np.asarray(gb2))

    ids_e, w_e, counts, elig = [], [], [], []
    for e in range(E):
        mask = (top2 == e)
        ids = np.nonzero(mask.any(axis=1))[0]
        w = wts[mask]
        o = np.argsort(w, kind="stable")
        ids_e.append(ids[o]); w_e.append(w[o])
        counts.append(len(ids))
        elig.append(int((w <= THETA).sum()))

    slots, n = _solve(counts, elig)
    slot_key = tuple(slots)

    ptr8 = [0] * E
    used8 = [0] * E
    for e in range(E):
        cap8 = sum(sz * n[e][i] for i, (t, sz) in enumerate(slots) if t == "8")
        used8[e] = min(cap8, elig[e])
    ptr16 = [used8[e] for e in range(E)]

    blocks = {}
    for i, (t, sz) in enumerate(slots):
        copy = 0
        for e in range(E):
            for _ in range(n[e][i]):
                if t == "8":
                    take = max(min(sz, used8[e] - ptr8[e]), 0)
                    sel = slice(ptr8[e], ptr8[e] + take)
                    ptr8[e] += take
                else:
                    take = max(min(sz, counts[e] - ptr16[e]), 0)
                    sel = slice(ptr16[e], ptr16[e] + take)
                    ptr16[e] += take
                blocks[(i, copy)] = (e, ids_e[e][sel], w_e[e][sel])
                copy += 1
        while copy < NCORES:
            blocks[(i, copy)] = (0, np.empty(0, np.int64),
                                 np.empty(0, np.float32))
            copy += 1
    for e in range(E):
        assert ptr16[e] >= counts[e], (e, ptr16[e], counts[e])

    w1p16, w1p8, w2p, b1p = {}, {}, {}, {}

    def get_w16(e):
        if e not in w1p16:
            w1p16[e] = np.ascontiguousarray(
                W1[e].reshape(KT1, 128, HT, 128).transpose(2, 1, 0, 3)
            ).reshape(HT, 128, KT1 * 128).astype(np.float16)
        return w1p16[e]

    def get_w8(e):
        if e not in w1p8:
            w1p8[e] = np.ascontiguousarray(
                (W1[e] * W1_SCALE).reshape(KT1, 128, HT, 128)
                .transpose(2, 1, 0, 3)
            ).astype(ml_dtypes.float8_e4m3)
        return w1p8[e]

    def get_w2(e):
        if e not in w2p:
            w2p[e] = np.ascontiguousarray(
                W2[e].reshape(KT2, 128, OT, 128).transpose(2, 1, 0, 3)
            ).reshape(OT, 128, KT2 * 128).astype(np.float16)
        return w2p[e]

    def get_b1(e):
        if e not in b1p:
            b1p[e] = np.ascontiguousarray(b1[e].reshape(HT, 128).T)
        return b1p[e]

    in_maps = [dict() for _ in range(NCORES)]
    for (i, copy), (e, ids, w) in blocks.items():
        t, sz = slots[i]
        m = in_maps[copy]
        nt = len(ids)
        xb = np.zeros((sz, IN_DIM), np.float32)
        if nt:
            xb[:nt] = X[ids]
        if t == "16":
            m[f"xt_{i}"] = _pack_x16(xb)
            m[f"w1_{i}"] = get_w16(e)
        else:
            m[f"xt_{i}"] = _pack_x8(xb)
            m[f"w1_{i}"] = get_w8(e)
        m[f"w2_{i}"] = get_w2(e)
        m[f"b1_{i}"] = get_b1(e)

    nc = _get_nc(slot_key)
    trace = bool(int(os.environ.get("KERNEL_TRACE", "0")))
    res = run_bass_kernel_spmd(nc, in_maps, list(range(NCORES)), trace=trace)
    LAST_RESULT = res

    out = np.zeros((B, OUT_DIM), np.float32)
    for (i, copy), (e, ids, w) in blocks.items():
        if not len(ids):
            continue
        t, sz = slots[i]
        yt = np.asarray(res.results[copy][f"yt_{i}"])
        y = yt.transpose(2, 0, 1).reshape(sz, OUT_DIM)[:len(ids)]
        out[ids] += w[:, None] * (y + b2[e][None, :])
    return out

